# revision 19
# baseline (speedup 1.0000x reference)
"""Multi-head attention Trainium2 kernel.

B=4, S=1024, D=1024, H=16, hd=64, f32 reference. 8 NeuronCores:
core c handles batch b=c//2, head-group g=c%2 (8 heads each) —
tensor-parallel over heads within a batch; the host sums the two
partial output projections per batch (the "all-reduce" of the
sharding hint) and adds bo.

All matmul operands are staged bf16 on the host (x, Wq/Wk/Wv, Wo, the
0/1 mask, a pre-broadcast V-bias panel); psum accumulation stays f32.
The 1/sqrt(hd) scale is folded into Wq/bq on the host.

Device dataflow (per core), everything feature-major so there are no
on-device transposes:
  qT[c,s] = sum_i Wq[i,c] xT[i,s] + bq          (lhsT=Wq tile, rhs=xT)
  kT      = k_raw + bk
  V[s,c]  = sum_i xT[i,s] Wv[i,c] + bv          (token-major; Wv is
            augmented with a ones column, so the softmax denominator
            falls out of the PV matmul as row 64)
  ST[k,q] = kT.T @ qT          (scores transposed, 2 heads row-tiled
                                via tile_position)
  PT      = exp(ST) * maskT    (ACT exp psum->sbuf bf16, then 2x-mode
                                DVE multiplicative bf16 mask; softmax
                                max-subtraction unnecessary: scaled
                                scores are ~N(0,1))
  valsT_aug[65,q] = sum over k-tiles (lhsT=V_aug[k,65], rhs=PT[k,q])
  vals    = valsT * (1/denom)
  out_partial[q,n] = vals.T @ Wo_rows  (bf16 partials; host upcasts)

Scheduling: the PV matmul stream runs one k-tile behind the
scores/exp/mask stream (the PE always has score matmuls in flight
while exp/mask catch up); each head-pair's psum accumulators are
copied to SBUF right after their last accumulation (split ACT/DVE, to
free banks 4-7 for the next pair quickly), and the
reciprocal/broadcast/normalize chain is emitted in chunks woven into
the NEXT pair's k-loop so it never backs up the DVE mask stream; the
[1,S]->[64,S] denominator broadcast runs on the otherwise idle GPSIMD
engine. Per-rep tiles are double-buffered so consecutive reps
pipeline (rep r+1's DMA + QKV matmuls overlap rep r's attention).
"""

import numpy as np

import concourse.bacc as bacc
import concourse.mybir as mybir
import concourse.tile as tile
from concourse import bass_utils
from concourse.alu_op_type import AluOpType

F32 = mybir.dt.float32
BF16 = mybir.dt.bfloat16
AF = mybir.ActivationFunctionType

B, S, D, H, HD = 4, 1024, 1024, 16, 64
NCORES = 8
HPC = 8            # heads per core
HAUG = HD + 1      # 65: V columns per head incl. ones column
VW = HPC * HAUG    # 520


def build_kernel(debug=False, krep=1, **_ignored):
    nc = bacc.Bacc(trn_type="TRN2", target_bir_lowering=False, debug=False,
                   num_devices=NCORES)

    xT = nc.dram_tensor("xT", [D, S], BF16, kind="ExternalInput").ap()
    maskT = nc.dram_tensor("maskT", [S, S], BF16, kind="ExternalInput").ap()
    wq = nc.dram_tensor("wq", [D, 512], BF16, kind="ExternalInput").ap()
    wk = nc.dram_tensor("wk", [D, 512], BF16, kind="ExternalInput").ap()
    wv = nc.dram_tensor("wv", [D, VW], BF16, kind="ExternalInput").ap()
    bq = nc.dram_tensor("bq", [512], F32, kind="ExternalInput").ap()
    bk = nc.dram_tensor("bk", [512], F32, kind="ExternalInput").ap()
    bvb_d = nc.dram_tensor("bvb", [128, VW], F32, kind="ExternalInput").ap()
    wo = nc.dram_tensor("wo", [512, S], BF16, kind="ExternalInput").ap()
    out = nc.dram_tensor("out", [S, S], BF16, kind="ExternalOutput").ap()
    if debug:
        d_q = nc.dram_tensor("d_q", [512, S], F32, kind="ExternalOutput").ap()
        d_k = nc.dram_tensor("d_k", [512, S], F32, kind="ExternalOutput").ap()
        d_vals = nc.dram_tensor("d_vals", [512, S], F32, kind="ExternalOutput").ap()

    PTAG = [f"T{i}" for i in range(8)]   # eight 1-bank psum slots

    with tile.TileContext(nc) as tc:
        with (
            tc.tile_pool(name="persist", bufs=1) as P,
            tc.tile_pool(name="ring", bufs=2) as R,
            tc.tile_pool(name="psum", bufs=1, space="PSUM") as PP,
        ):
            def ptile(shape, idx, name):
                return PP.tile(shape, F32, tag=PTAG[idx & 7],
                               name=f"{name}{idx & 7}")

            # ---- persistent constants ----
            bq_t = P.tile([128, 4], F32, tag="bq", name="bq")
            bk_t = P.tile([128, 4], F32, tag="bk", name="bk")
            bvb = P.tile([128, VW], F32, tag="bvb", name="bvb")
            nc.sync.dma_start(bq_t[:], bq.rearrange("(t p) -> p t", p=128))
            nc.sync.dma_start(bk_t[:], bk.rearrange("(t p) -> p t", p=128))
            nc.sync.dma_start(bvb[:], bvb_d)

            for rep in range(krep):
                # ---- double-buffered per-rep tiles ----
                mN = [R.tile([128, S], BF16, tag=f"mN{t}", name=f"mN{t}",
                             bufs=1) for t in range(8)]
                xT_t = [R.tile([128, S], BF16, tag=f"xT{i}", name=f"xT{i}")
                        for i in range(8)]
                wq_t = [R.tile([128, 512], BF16, tag=f"wq{i}", name=f"wq{i}")
                        for i in range(8)]
                wk_t = [R.tile([128, 512], BF16, tag=f"wk{i}", name=f"wk{i}")
                        for i in range(8)]
                wv_t = [R.tile([128, VW], BF16, tag=f"wv{i}", name=f"wv{i}")
                        for i in range(8)]
                wo_t = [R.tile([128, S], BF16, tag=f"wo{t}", name=f"wo{t}",
                               bufs=1) for t in range(4)]
                qT = [R.tile([128, S], BF16, tag=f"qT{t}", name=f"qT{t}")
                      for t in range(4)]
                kT = [R.tile([128, S], BF16, tag=f"kT{t}", name=f"kT{t}")
                      for t in range(4)]
                vA = [R.tile([128, VW], BF16, tag=f"vA{t}", name=f"vA{t}")
                      for t in range(8)]
                vals = [R.tile([128, S], BF16, tag=f"vals{p}",
                               name=f"vals{p}", bufs=1) for p in range(4)]

                for i in range(8):
                    rs = slice(i * 128, (i + 1) * 128)
                    nc.sync.dma_start(mN[i][:], maskT[rs, :])
                    nc.sync.dma_start(xT_t[i][:], xT[rs, :])
                    nc.sync.dma_start(wq_t[i][:], wq[rs, :])
                    nc.sync.dma_start(wk_t[i][:], wk[rs, :])
                    nc.sync.dma_start(wv_t[i][:], wv[rs, :])
                for t in range(4):
                    nc.sync.dma_start(wo_t[t][:], wo[t * 128:(t + 1) * 128, :])

                # ---- stage 1: QKV projections ----
                with nc.allow_low_precision(reason="matmul feed"):
                    # V token-major with augmented ones column
                    for st in range(8):
                        ts_ = slice(st * 128, (st + 1) * 128)
                        for h2 in range(2):
                            cs = slice(h2 * 260, (h2 + 1) * 260)
                            pv = ptile([128, 260], 4 * h2 + (st & 3), "pv")
                            for i in range(8):
                                nc.tensor.matmul(pv[:], xT_t[i][:, ts_],
                                                 wv_t[i][:, cs],
                                                 start=(i == 0), stop=(i == 7))
                            nc.vector.tensor_tensor(vA[st][:, cs], pv[:],
                                                    bvb[:, cs], AluOpType.add)

                    # q/k feature-major: psum[c,s] accumulated over i-tiles
                    for t in range(4):
                        cs = slice(t * 128, (t + 1) * 128)
                        for sh in range(2):
                            ss = slice(sh * 512, (sh + 1) * 512)
                            pq = ptile([128, 512], 2 * sh + (t & 1), "pq")
                            pk = ptile([128, 512], 4 + 2 * sh + (t & 1), "pk")
                            for i in range(8):
                                nc.tensor.matmul(pq[:], wq_t[i][:, cs],
                                                 xT_t[i][:, ss],
                                                 start=(i == 0), stop=(i == 7))
                            for i in range(8):
                                nc.tensor.matmul(pk[:], wk_t[i][:, cs],
                                                 xT_t[i][:, ss],
                                                 start=(i == 0), stop=(i == 7))
                            nc.vector.tensor_scalar(qT[t][:, ss], pq[:],
                                                    bq_t[:, t:t + 1], None,
                                                    AluOpType.add)
                            nc.vector.tensor_scalar(kT[t][:, ss], pk[:],
                                                    bk_t[:, t:t + 1], None,
                                                    AluOpType.add)

                if debug:
                    for t in range(4):
                        nc.sync.dma_start(d_q[t * 128:(t + 1) * 128, 0:512],
                                          qT[t][:].bitcast(F32))
                        nc.sync.dma_start(d_k[t * 128:(t + 1) * 128, 0:512],
                                          kT[t][:].bitcast(F32))

                # ---- stage 2: attention per head-pair ----
                den = R.tile([1, S], F32, tag="den", name="den", bufs=1)
                rec = R.tile([1, S], F32, tag="rec", name="rec", bufs=1)

                def norm_chunks(p, vraw, bcs_l):
                    """Pair p's normalization, split into 4 closures that
                    get woven into the next pair's k-loop so the DVE mask
                    stream never backs up: per hh, (stage den f32 +
                    reciprocal + GPSIMD broadcast), then the multiplies."""
                    def chain(hh):
                        for qh in range(2):
                            qs = slice(qh * 512, (qh + 1) * 512)
                            nc.vector.tensor_copy(den[0:1, qs],
                                                  vraw[hh][qh][64:65, :])
                        nc.vector.reciprocal_approx_fast(rec[:], den[:])
                        for qh in range(2):
                            qs = slice(qh * 512, (qh + 1) * 512)
                            nc.gpsimd.partition_broadcast(
                                bcs_l[2 * hh + qh][:], rec[0:1, qs])

                    def mults(hh):
                        with nc.allow_low_precision(reason="matmul feed"):
                            for qh in range(2):
                                qs = slice(qh * 512, (qh + 1) * 512)
                                nc.vector.tensor_tensor(
                                    vals[p][hh * 64:(hh + 1) * 64, qs],
                                    vraw[hh][qh][0:64, :],
                                    bcs_l[2 * hh + qh][:], AluOpType.mult)

                    return [lambda: chain(0), lambda: mults(0),
                            lambda: chain(1), lambda: mults(1)]

                pending = []
                for p in range(4):   # head pair
                    vpsf = [[ptile([HAUG, 512], 4 + 2 * hh + qh, "vps")
                             for qh in range(2)] for hh in range(2)]
                    # PV runs LAG k-tiles behind scores/exp/mask
                    LAG = 2
                    pts = {}
                    for kt in range(8 + LAG):
                        ks = slice(kt * 128, (kt + 1) * 128)
                        for qh in range(2):
                            qs = slice(qh * 512, (qh + 1) * 512)
                            for hh in range(2):
                                h = 2 * p + hh
                                ds = slice(hh * 64, (hh + 1) * 64)
                                if kt < 8:
                                    stp = ptile([128, 512], 2 * hh + qh,
                                                "stp")
                                    nc.tensor.matmul(stp[:], kT[p][ds, ks],
                                                     qT[p][ds, qs],
                                                     start=True, stop=True,
                                                     tile_position=(hh * 64,
                                                                    0))
                                    pt = R.tile([128, 512], BF16, tag="pt",
                                                name="pt", bufs=12)
                                    nc.scalar.activation(pt[:], stp[:],
                                                         AF.Exp)
                                    with nc.allow_low_precision(
                                            reason="matmul feed"):
                                        nc.vector.tensor_tensor(
                                            pt[:], pt[:], mN[kt][:, qs],
                                            AluOpType.mult)
                                    pts[(kt, qh, hh)] = pt
                                if kt >= LAG:
                                    nc.tensor.matmul(
                                        vpsf[hh][qh][:],
                                        vA[kt - LAG][:,
                                                     h * HAUG:(h + 1) * HAUG],
                                        pts.pop((kt - LAG, qh, hh)),
                                        start=(kt == LAG),
                                        stop=(kt == 7 + LAG))
                        # weave the previous pair's normalization in
                        if kt in (2, 4, 6, 8) and pending:
                            pending.pop(0)()

                    # Copy-first: dump each vpsf psum tile (incl. den row
                    # 64) to SBUF bf16 immediately so banks 4-7 free up
                    # for the next pair; copies split ACT/DVE to halve
                    # the engine-FIFO insertion delay.
                    vraw = [[R.tile([HAUG, 512], BF16, tag=f"vraw{2*hh+qh}",
                                    name="vraw", bufs=2)
                             for qh in range(2)] for hh in range(2)]
                    with nc.allow_low_precision(reason="matmul feed"):
                        for qh in range(2):
                            nc.scalar.activation(vraw[0][qh][:],
                                                 vpsf[0][qh][:], AF.Identity)
                            nc.vector.tensor_copy(vraw[1][qh][:],
                                                  vpsf[1][qh][:])
                    bcs_l = [R.tile([64, 512], F32, tag="bcs", name="bcs",
                                    bufs=4) for _ in range(4)]
                    if p == 3:
                        # Last pair: no next k-loop to weave into, and the
                        # output projection is waiting. Stage the dens on
                        # the (now idle) ACT engine straight from PSUM and
                        # run the two hh chains back to back.
                        for hh in range(2):
                            for qh in range(2):
                                qs = slice(qh * 512, (qh + 1) * 512)
                                nc.scalar.activation(den[0:1, qs],
                                                     vpsf[hh][qh][64:65, :],
                                                     AF.Identity)
                            nc.vector.reciprocal_approx_fast(rec[:], den[:])
                            for qh in range(2):
                                qs = slice(qh * 512, (qh + 1) * 512)
                                nc.gpsimd.partition_broadcast(
                                    bcs_l[2 * hh + qh][:], rec[0:1, qs])
                            with nc.allow_low_precision(
                                    reason="matmul feed"):
                                for qh in range(2):
                                    qs = slice(qh * 512, (qh + 1) * 512)
                                    nc.vector.tensor_tensor(
                                        vals[p][hh * 64:(hh + 1) * 64, qs],
                                        vraw[hh][qh][0:64, :],
                                        bcs_l[2 * hh + qh][:],
                                        AluOpType.mult)
                    else:
                        pending.extend(norm_chunks(p, vraw, bcs_l))

                if debug:
                    for pi in range(4):
                        nc.sync.dma_start(d_vals[pi * 128:(pi + 1) * 128,
                                                 0:512],
                                          vals[pi][:].bitcast(F32))

                # ---- stage 3: output projection ----
                # Two-phase accumulation per wave of 8 psum banks: the
                # pi<3 matmuls run while head-pair 3 is still normalizing;
                # the pi=3 matmuls (which need vals[3]) come last.
                for wave in range(2):
                    po_w = {}
                    for qt in range(wave * 4, wave * 4 + 4):
                        qs = slice(qt * 128, (qt + 1) * 128)
                        for nh in range(2):
                            ns = slice(nh * 512, (nh + 1) * 512)
                            po = ptile([128, 512], 2 * qt + nh, "po")
                            po_w[(qt, nh)] = po
                            for pi in range(3):
                                nc.tensor.matmul(po[:], vals[pi][:, qs],
                                                 wo_t[pi][:, ns],
                                                 start=(pi == 0), stop=False)
                    for qt in range(wave * 4, wave * 4 + 4):
                        qs = slice(qt * 128, (qt + 1) * 128)
                        ot = R.tile([128, S], BF16, tag="ot", name="ot",
                                    bufs=2)
                        for nh in range(2):
                            ns = slice(nh * 512, (nh + 1) * 512)
                            po = po_w[(qt, nh)]
                            nc.tensor.matmul(po[:], vals[3][:, qs],
                                             wo_t[3][:, ns],
                                             start=False, stop=True)
                            with nc.allow_low_precision(
                                    reason="matmul feed"):
                                nc.scalar.activation(ot[:, ns], po[:],
                                                     AF.Identity)
                        nc.sync.dma_start(out[qs, :], ot[:])

    nc.compile()
    return nc


_NC_CACHE = {}


def _get_nc():
    if "nc" not in _NC_CACHE:
        _NC_CACHE["nc"] = build_kernel()
    return _NC_CACHE["nc"]


def _bf16(a):
    import jax.numpy as jnp
    return np.asarray(jnp.asarray(a, dtype=jnp.bfloat16))


def shard_inputs(x, mask, Wqkv, bqkv, Wo, bo):
    """Per-core input dicts. Layout/slicing/dtype staging only."""
    x = np.asarray(x, dtype=np.float32)
    mask = np.asarray(mask, dtype=np.int32)
    Wqkv = np.asarray(Wqkv, dtype=np.float32)
    bqkv = np.asarray(bqkv, dtype=np.float32)
    Wo = np.asarray(Wo, dtype=np.float32)

    scale = 1.0 / np.sqrt(HD)
    Wr = Wqkv.reshape(D, H, 3, HD)
    br = bqkv.reshape(H, 3, HD)
    in_maps = []
    for c in range(NCORES):
        b, g = c // 2, c % 2
        hs = slice(g * HPC, (g + 1) * HPC)
        wv_aug = np.zeros((D, HPC, HAUG), dtype=np.float32)
        wv_aug[:, :, :HD] = Wr[:, hs, 2, :]
        bv_aug = np.zeros((HPC, HAUG), dtype=np.float32)
        bv_aug[:, :HD] = br[hs, 2, :]
        bv_aug[:, HD] = 1.0
        bvb = np.tile(bv_aug.reshape(1, VW), (128, 1))
        in_maps.append({
            "xT": _bf16(x[b].T),
            "maskT": _bf16(mask[b].T),
            "wq": _bf16(Wr[:, hs, 0, :].reshape(D, 512) * scale),
            "wk": _bf16(Wr[:, hs, 1, :].reshape(D, 512)),
            "wv": _bf16(wv_aug.reshape(D, VW)),
            "bq": np.ascontiguousarray(br[hs, 0, :].reshape(512) * scale),
            "bk": np.ascontiguousarray(br[hs, 1, :].reshape(512)),
            "bvb": np.ascontiguousarray(bvb),
            "wo": _bf16(Wo[g * 512:(g + 1) * 512, :]),
        })
    return in_maps


def combine_outputs(results, bo):
    bo = np.asarray(bo, dtype=np.float32)
    out = np.empty((B, S, D), dtype=np.float32)
    for b in range(B):
        out[b] = (np.asarray(results[2 * b]["out"], dtype=np.float32)
                  + np.asarray(results[2 * b + 1]["out"], dtype=np.float32)
                  + bo)
    return out


def kernel(x, mask, Wqkv, bqkv, Wo, bo):
    nc = _get_nc()
    in_maps = shard_inputs(x, mask, Wqkv, bqkv, Wo, bo)
    res = bass_utils.run_bass_kernel_spmd(nc, in_maps,
                                          core_ids=list(range(NCORES)))
    return combine_outputs(res.results, bo)


# revision 22
# speedup vs baseline: 1.0489x; 1.0489x over previous
"""Multi-head attention Trainium2 kernel.

B=4, S=1024, D=1024, H=16, hd=64, f32 reference. 8 NeuronCores:
core c handles batch b=c//2, head-group g=c%2 (8 heads each) —
tensor-parallel over heads within a batch; the host sums the two
partial output projections per batch (the "all-reduce" of the
sharding hint) and adds bo.

All matmul operands are staged bf16 on the host (x, Wq/Wk/Wv, Wo, the
0/1 mask, a pre-broadcast V-bias panel); psum accumulation stays f32.
The 1/sqrt(hd) scale is folded into Wq/bq on the host.

Device dataflow (per core), everything feature-major so there are no
on-device transposes:
  qT[c,s] = sum_i Wq[i,c] xT[i,s] + bq          (lhsT=Wq tile, rhs=xT)
  kT      = k_raw + bk
  V[s,c]  = sum_i xT[i,s] Wv[i,c] + bv          (token-major; Wv is
            augmented with a ones column, so the softmax denominator
            falls out of the PV matmul as row 64)
  ST[k,q] = kT.T @ qT          (scores transposed, 2 heads row-tiled
                                via tile_position)
  PT      = exp(ST) * maskT    (ACT exp psum->sbuf bf16, then 2x-mode
                                DVE multiplicative bf16 mask; softmax
                                max-subtraction unnecessary: scaled
                                scores are ~N(0,1))
  valsT_aug[65,q] = sum over k-tiles (lhsT=V_aug[k,65], rhs=PT[k,q])
  vals    = valsT * (1/denom)
  out_partial[q,n] = vals.T @ Wo_rows  (bf16 partials; host upcasts)

Scheduling: the PV matmul stream runs one k-tile behind the
scores/exp/mask stream (the PE always has score matmuls in flight
while exp/mask catch up); each head-pair's psum accumulators are
copied to SBUF right after their last accumulation (split ACT/DVE, to
free banks 4-7 for the next pair quickly), and the
reciprocal/broadcast/normalize chain is emitted in chunks woven into
the NEXT pair's k-loop so it never backs up the DVE mask stream; the
[1,S]->[64,S] denominator broadcast runs on the otherwise idle GPSIMD
engine. Per-rep tiles are double-buffered so consecutive reps
pipeline (rep r+1's DMA + QKV matmuls overlap rep r's attention).
"""

import numpy as np

import concourse.bacc as bacc
import concourse.mybir as mybir
import concourse.tile as tile
from concourse import bass_utils
from concourse.alu_op_type import AluOpType

F32 = mybir.dt.float32
BF16 = mybir.dt.bfloat16
AF = mybir.ActivationFunctionType

B, S, D, H, HD = 4, 1024, 1024, 16, 64
NCORES = 8
HPC = 8            # heads per core
HAUG = HD + 1      # 65: V columns per head incl. ones column
VW = HPC * HAUG    # 520


def build_kernel(debug=False, krep=1, **_ignored):
    nc = bacc.Bacc(trn_type="TRN2", target_bir_lowering=False, debug=False,
                   num_devices=NCORES)

    xT = nc.dram_tensor("xT", [D, S], BF16, kind="ExternalInput").ap()
    maskT = nc.dram_tensor("maskT", [S, S], BF16, kind="ExternalInput").ap()
    wq = nc.dram_tensor("wq", [D, 512], BF16, kind="ExternalInput").ap()
    wk = nc.dram_tensor("wk", [D, 512], BF16, kind="ExternalInput").ap()
    wv = nc.dram_tensor("wv", [D, VW], BF16, kind="ExternalInput").ap()
    bq = nc.dram_tensor("bq", [512], F32, kind="ExternalInput").ap()
    bk = nc.dram_tensor("bk", [512], F32, kind="ExternalInput").ap()
    bvb_d = nc.dram_tensor("bvb", [128, VW], F32, kind="ExternalInput").ap()
    wo = nc.dram_tensor("wo", [512, S], BF16, kind="ExternalInput").ap()
    out = nc.dram_tensor("out", [S, S], BF16, kind="ExternalOutput").ap()
    if debug:
        d_q = nc.dram_tensor("d_q", [512, S], F32, kind="ExternalOutput").ap()
        d_k = nc.dram_tensor("d_k", [512, S], F32, kind="ExternalOutput").ap()
        d_vals = nc.dram_tensor("d_vals", [512, S], F32, kind="ExternalOutput").ap()

    PTAG = [f"T{i}" for i in range(8)]   # eight 1-bank psum slots

    with tile.TileContext(nc) as tc:
        with (
            tc.tile_pool(name="persist", bufs=1) as P,
            tc.tile_pool(name="ring", bufs=2) as R,
            tc.tile_pool(name="psum", bufs=1, space="PSUM") as PP,
        ):
            def ptile(shape, idx, name):
                return PP.tile(shape, F32, tag=PTAG[idx & 7],
                               name=f"{name}{idx & 7}")

            # ---- persistent constants ----
            bq_t = P.tile([128, 4], F32, tag="bq", name="bq")
            bk_t = P.tile([128, 4], F32, tag="bk", name="bk")
            bvb = P.tile([128, VW], F32, tag="bvb", name="bvb")
            nc.sync.dma_start(bq_t[:], bq.rearrange("(t p) -> p t", p=128))
            nc.sync.dma_start(bk_t[:], bk.rearrange("(t p) -> p t", p=128))
            nc.sync.dma_start(bvb[:], bvb_d)

            def alloc_rep():
                t = {}
                t["mN"] = [R.tile([128, S], BF16, tag=f"mN{i}", name=f"mN{i}",
                                  bufs=1) for i in range(8)]
                t["xT"] = [R.tile([128, S], BF16, tag=f"xT{i}", name=f"xT{i}")
                           for i in range(8)]
                t["wq"] = [R.tile([128, 512], BF16, tag=f"wq{i}",
                                  name=f"wq{i}") for i in range(8)]
                t["wk"] = [R.tile([128, 512], BF16, tag=f"wk{i}",
                                  name=f"wk{i}") for i in range(8)]
                t["wv"] = [R.tile([128, VW], BF16, tag=f"wv{i}",
                                  name=f"wv{i}") for i in range(8)]
                t["wo"] = [R.tile([128, S], BF16, tag=f"wo{i}",
                                  name=f"wo{i}", bufs=1) for i in range(4)]
                t["qT"] = [R.tile([128, S], BF16, tag=f"qT{i}",
                                  name=f"qT{i}") for i in range(4)]
                t["kT"] = [R.tile([128, S], BF16, tag=f"kT{i}",
                                  name=f"kT{i}") for i in range(4)]
                t["vA"] = [R.tile([128, VW], BF16, tag=f"vA{i}",
                                  name=f"vA{i}") for i in range(8)]
                t["vals"] = [R.tile([128, S], BF16, tag=f"vals{i}",
                                    name=f"vals{i}", bufs=1)
                             for i in range(4)]
                return t

            def emit_dmas(t):
                # xT+wv first: the first woven V-projection chunks need
                # them; masks last (only needed by that rep's attention)
                for i in range(8):
                    rs = slice(i * 128, (i + 1) * 128)
                    nc.sync.dma_start(t["xT"][i][:], xT[rs, :])
                for i in range(8):
                    rs = slice(i * 128, (i + 1) * 128)
                    nc.sync.dma_start(t["wv"][i][:], wv[rs, :])
                for i in range(8):
                    rs = slice(i * 128, (i + 1) * 128)
                    nc.sync.dma_start(t["wq"][i][:], wq[rs, :])
                    nc.sync.dma_start(t["wk"][i][:], wk[rs, :])
                for i in range(4):
                    nc.sync.dma_start(t["wo"][i][:],
                                      wo[i * 128:(i + 1) * 128, :])
                for i in range(8):
                    rs = slice(i * 128, (i + 1) * 128)
                    nc.sync.dma_start(t["mN"][i][:], maskT[rs, :])

            def stage1_chunks(t, tag_iter):
                """QKV projection as 32 independent closures, each one
                psum-accumulation group on a reserved bank (tags T2/T3),
                so they can be woven into the previous rep's exp-paced
                attention loop to fill PE bubbles."""
                chunks = []

                def v_group(st, h2):
                    def go():
                        cs = slice(h2 * 260, (h2 + 1) * 260)
                        ts_ = slice(st * 128, (st + 1) * 128)
                        pv = ptile([128, 260], next(tag_iter), "pv")
                        with nc.allow_low_precision(reason="matmul feed"):
                            for i in range(8):
                                nc.tensor.matmul(pv[:], t["xT"][i][:, ts_],
                                                 t["wv"][i][:, cs],
                                                 start=(i == 0),
                                                 stop=(i == 7))
                            nc.vector.tensor_tensor(t["vA"][st][:, cs],
                                                    pv[:], bvb[:, cs],
                                                    AluOpType.add)
                    return go

                def qk_group(which, tt, sh):
                    def go():
                        cs = slice(tt * 128, (tt + 1) * 128)
                        ss = slice(sh * 512, (sh + 1) * 512)
                        w = t["wq"] if which == "q" else t["wk"]
                        dst = t["qT"] if which == "q" else t["kT"]
                        bias = bq_t if which == "q" else bk_t
                        pq = ptile([128, 512], next(tag_iter), "pq")
                        with nc.allow_low_precision(reason="matmul feed"):
                            for i in range(8):
                                nc.tensor.matmul(pq[:], w[i][:, cs],
                                                 t["xT"][i][:, ss],
                                                 start=(i == 0),
                                                 stop=(i == 7))
                            nc.vector.tensor_scalar(dst[tt][:, ss], pq[:],
                                                    bias[:, tt:tt + 1],
                                                    None, AluOpType.add)
                    return go

                for st in range(8):
                    for h2 in range(2):
                        chunks.append(v_group(st, h2))
                for tt in range(4):
                    for sh in range(2):
                        chunks.append(qk_group("q", tt, sh))
                        chunks.append(qk_group("k", tt, sh))
                return chunks

            def tag23():
                i = 0
                while True:
                    yield 2 + (i & 1)
                    i += 1

            def tag07():
                i = 0
                while True:
                    yield i & 7
                    i += 1

            # ---- software-pipelined rep loop: rep r's attention weaves
            # in rep r+1's QKV projection chunks (psum tags T2/T3) so the
            # exp-paced attention phase keeps the PE busy ----
            den = R.tile([1, S], F32, tag="den", name="den", bufs=1)
            rec = R.tile([1, S], F32, tag="rec", name="rec", bufs=1)

            def norm_chunks(t, p, vraw, bcs_l):
                """Pair p's normalization, split into 4 closures woven
                into the next pair's k-loop so the DVE mask stream never
                backs up: per hh, (stage den f32 + reciprocal + GPSIMD
                broadcast), then the multiplies."""
                def chain(hh):
                    for qh in range(2):
                        qs = slice(qh * 512, (qh + 1) * 512)
                        nc.vector.tensor_copy(den[0:1, qs],
                                              vraw[hh][qh][64:65, :])
                    nc.vector.reciprocal_approx_fast(rec[:], den[:])
                    for qh in range(2):
                        qs = slice(qh * 512, (qh + 1) * 512)
                        nc.gpsimd.partition_broadcast(
                            bcs_l[2 * hh + qh][:], rec[0:1, qs])

                def mults(hh):
                    with nc.allow_low_precision(reason="matmul feed"):
                        for qh in range(2):
                            qs = slice(qh * 512, (qh + 1) * 512)
                            nc.vector.tensor_tensor(
                                t["vals"][p][hh * 64:(hh + 1) * 64, qs],
                                vraw[hh][qh][0:64, :],
                                bcs_l[2 * hh + qh][:], AluOpType.mult)

                return [lambda: chain(0), lambda: mults(0),
                        lambda: chain(1), lambda: mults(1)]

            def attention(t, weave):
                """Attention stage for this rep's tiles. `weave` is a list
                of closures (next rep's stage-1 groups) consumed one per
                k-tile slot to fill PE bubbles."""
                pending = []
                qT, kT, vA, mN = t["qT"], t["kT"], t["vA"], t["mN"]
                for p in range(4):   # head pair
                    vpsf = [[ptile([HAUG, 512], 4 + 2 * hh + qh, "vps")
                             for qh in range(2)] for hh in range(2)]
                    LAG = 2          # PV runs LAG k-tiles behind scores
                    pts = {}
                    for kt in range(8 + LAG):
                        if weave and p >= 1:
                            weave.pop(0)()
                        ks = slice(kt * 128, (kt + 1) * 128)
                        for qh in range(2):
                            qs = slice(qh * 512, (qh + 1) * 512)
                            for hh in range(2):
                                h = 2 * p + hh
                                ds = slice(hh * 64, (hh + 1) * 64)
                                if kt < 8:
                                    stp = ptile([128, 512], 2 * hh + qh,
                                                "stp")
                                    nc.tensor.matmul(stp[:], kT[p][ds, ks],
                                                     qT[p][ds, qs],
                                                     start=True, stop=True,
                                                     tile_position=(hh * 64,
                                                                    0))
                                    pt = R.tile([128, 512], BF16, tag="pt",
                                                name="pt", bufs=12)
                                    nc.scalar.activation(pt[:], stp[:],
                                                         AF.Exp)
                                    with nc.allow_low_precision(
                                            reason="matmul feed"):
                                        nc.vector.tensor_tensor(
                                            pt[:], pt[:], mN[kt][:, qs],
                                            AluOpType.mult)
                                    pts[(kt, qh, hh)] = pt
                                if kt >= LAG:
                                    nc.tensor.matmul(
                                        vpsf[hh][qh][:],
                                        vA[kt - LAG][:,
                                                     h * HAUG:(h + 1) * HAUG],
                                        pts.pop((kt - LAG, qh, hh)),
                                        start=(kt == LAG),
                                        stop=(kt == 7 + LAG))
                        # weave the previous pair's normalization in
                        if kt in (2, 4, 6, 8) and pending:
                            pending.pop(0)()

                    # Copy-first: dump each vpsf psum tile (incl. den row
                    # 64) to SBUF bf16 immediately so banks 4-7 free up
                    # for the next pair; copies split ACT/DVE to halve
                    # the engine-FIFO insertion delay.
                    vraw = [[R.tile([HAUG, 512], BF16, tag=f"vraw{2*hh+qh}",
                                    name="vraw", bufs=2)
                             for qh in range(2)] for hh in range(2)]
                    with nc.allow_low_precision(reason="matmul feed"):
                        for qh in range(2):
                            nc.scalar.activation(vraw[0][qh][:],
                                                 vpsf[0][qh][:], AF.Identity)
                            nc.vector.tensor_copy(vraw[1][qh][:],
                                                  vpsf[1][qh][:])
                    bcs_l = [R.tile([64, 512], F32, tag="bcs", name="bcs",
                                    bufs=4) for _ in range(4)]
                    if p == 3:
                        # Last pair: no next k-loop to weave into; stage
                        # the dens on the now-idle ACT engine straight
                        # from PSUM and run the chains back to back.
                        for hh in range(2):
                            for qh in range(2):
                                qs = slice(qh * 512, (qh + 1) * 512)
                                nc.scalar.activation(den[0:1, qs],
                                                     vpsf[hh][qh][64:65, :],
                                                     AF.Identity)
                            nc.vector.reciprocal_approx_fast(rec[:], den[:])
                            for qh in range(2):
                                qs = slice(qh * 512, (qh + 1) * 512)
                                nc.gpsimd.partition_broadcast(
                                    bcs_l[2 * hh + qh][:], rec[0:1, qs])
                            with nc.allow_low_precision(
                                    reason="matmul feed"):
                                for qh in range(2):
                                    qs = slice(qh * 512, (qh + 1) * 512)
                                    nc.vector.tensor_tensor(
                                        t["vals"][p][hh * 64:(hh + 1) * 64,
                                                     qs],
                                        vraw[hh][qh][0:64, :],
                                        bcs_l[2 * hh + qh][:],
                                        AluOpType.mult)
                    else:
                        pending.extend(norm_chunks(t, p, vraw, bcs_l))

            def outproj(t, weave):
                """Output projection in waves of 3/3/2 qt over psum tags
                T2..T7; within each wave the pi<3 matmuls come first so
                head-pair 3's normalization tail is covered. Leftover
                weave chunks fill the gaps."""
                vals, wo_t = t["vals"], t["wo"]
                for wave, qts in enumerate(([0, 1, 2], [3, 4, 5], [6, 7])):
                    po_w = {}
                    for wi, qt in enumerate(qts):
                        qs = slice(qt * 128, (qt + 1) * 128)
                        for nh in range(2):
                            ns = slice(nh * 512, (nh + 1) * 512)
                            po = ptile([128, 512], 2 + 2 * wi + nh, "po")
                            po_w[(qt, nh)] = po
                            for pi in range(3):
                                nc.tensor.matmul(po[:], vals[pi][:, qs],
                                                 wo_t[pi][:, ns],
                                                 start=(pi == 0), stop=False)
                    if weave:
                        weave.pop(0)()
                    for qt in qts:
                        qs = slice(qt * 128, (qt + 1) * 128)
                        ot = R.tile([128, S], BF16, tag="ot", name="ot",
                                    bufs=2)
                        for nh in range(2):
                            ns = slice(nh * 512, (nh + 1) * 512)
                            po = po_w[(qt, nh)]
                            nc.tensor.matmul(po[:], vals[3][:, qs],
                                             wo_t[3][:, ns],
                                             start=False, stop=True)
                            with nc.allow_low_precision(
                                    reason="matmul feed"):
                                nc.scalar.activation(ot[:, ns], po[:],
                                                     AF.Identity)
                        nc.sync.dma_start(out[qs, :], ot[:])

            # prologue: rep 0's tiles + full stage 1 inline
            cur = alloc_rep()
            emit_dmas(cur)
            for c in stage1_chunks(cur, tag07()):
                c()
            for rep in range(krep):
                if rep + 1 < krep:
                    nxt = alloc_rep()
                    emit_dmas(nxt)
                    weave = stage1_chunks(nxt, tag23())
                else:
                    nxt, weave = None, []
                attention(cur, weave)
                if debug:
                    for pi in range(4):
                        nc.sync.dma_start(
                            d_vals[pi * 128:(pi + 1) * 128, 0:512],
                            cur["vals"][pi][:].bitcast(F32))
                    for tt in range(4):
                        nc.sync.dma_start(d_q[tt * 128:(tt + 1) * 128, 0:512],
                                          cur["qT"][tt][:].bitcast(F32))
                        nc.sync.dma_start(d_k[tt * 128:(tt + 1) * 128, 0:512],
                                          cur["kT"][tt][:].bitcast(F32))
                outproj(cur, weave)
                while weave:
                    weave.pop(0)()
                cur = nxt

    nc.compile()
    return nc


_NC_CACHE = {}


def _get_nc():
    if "nc" not in _NC_CACHE:
        _NC_CACHE["nc"] = build_kernel()
    return _NC_CACHE["nc"]


def _bf16(a):
    import jax.numpy as jnp
    return np.asarray(jnp.asarray(a, dtype=jnp.bfloat16))


def shard_inputs(x, mask, Wqkv, bqkv, Wo, bo):
    """Per-core input dicts. Layout/slicing/dtype staging only."""
    x = np.asarray(x, dtype=np.float32)
    mask = np.asarray(mask, dtype=np.int32)
    Wqkv = np.asarray(Wqkv, dtype=np.float32)
    bqkv = np.asarray(bqkv, dtype=np.float32)
    Wo = np.asarray(Wo, dtype=np.float32)

    scale = 1.0 / np.sqrt(HD)
    Wr = Wqkv.reshape(D, H, 3, HD)
    br = bqkv.reshape(H, 3, HD)
    in_maps = []
    for c in range(NCORES):
        b, g = c // 2, c % 2
        hs = slice(g * HPC, (g + 1) * HPC)
        wv_aug = np.zeros((D, HPC, HAUG), dtype=np.float32)
        wv_aug[:, :, :HD] = Wr[:, hs, 2, :]
        bv_aug = np.zeros((HPC, HAUG), dtype=np.float32)
        bv_aug[:, :HD] = br[hs, 2, :]
        bv_aug[:, HD] = 1.0
        bvb = np.tile(bv_aug.reshape(1, VW), (128, 1))
        in_maps.append({
            "xT": _bf16(x[b].T),
            "maskT": _bf16(mask[b].T),
            "wq": _bf16(Wr[:, hs, 0, :].reshape(D, 512) * scale),
            "wk": _bf16(Wr[:, hs, 1, :].reshape(D, 512)),
            "wv": _bf16(wv_aug.reshape(D, VW)),
            "bq": np.ascontiguousarray(br[hs, 0, :].reshape(512) * scale),
            "bk": np.ascontiguousarray(br[hs, 1, :].reshape(512)),
            "bvb": np.ascontiguousarray(bvb),
            "wo": _bf16(Wo[g * 512:(g + 1) * 512, :]),
        })
    return in_maps


def combine_outputs(results, bo):
    bo = np.asarray(bo, dtype=np.float32)
    out = np.empty((B, S, D), dtype=np.float32)
    for b in range(B):
        out[b] = (np.asarray(results[2 * b]["out"], dtype=np.float32)
                  + np.asarray(results[2 * b + 1]["out"], dtype=np.float32)
                  + bo)
    return out


def kernel(x, mask, Wqkv, bqkv, Wo, bo):
    nc = _get_nc()
    in_maps = shard_inputs(x, mask, Wqkv, bqkv, Wo, bo)
    res = bass_utils.run_bass_kernel_spmd(nc, in_maps,
                                          core_ids=list(range(NCORES)))
    return combine_outputs(res.results, bo)


# revision 25
# speedup vs baseline: 1.1950x; 1.1392x over previous
"""Multi-head attention Trainium2 kernel.

B=4, S=1024, D=1024, H=16, hd=64, f32 reference. 8 NeuronCores:
core c handles batch b=c//2, head-group g=c%2 (8 heads each) —
tensor-parallel over heads within a batch; the host sums the two
partial output projections per batch (the "all-reduce" of the
sharding hint) and adds bo.

All matmul operands are staged bf16 on the host (x, Wq/Wk/Wv, Wo, the
0/1 mask, a pre-broadcast V-bias panel); psum accumulation stays f32.
The 1/sqrt(hd) scale is folded into Wq/bq on the host.

Device dataflow (per core), everything feature-major so there are no
on-device transposes:
  qT[c,s] = sum_i Wq[i,c] xT[i,s] + bq          (lhsT=Wq tile, rhs=xT)
  kT      = k_raw + bk
  V[s,c]  = sum_i xT[i,s] Wv[i,c] + bv          (token-major; Wv is
            augmented with a ones column, so the softmax denominator
            falls out of the PV matmul as row 64)
  ST[k,q] = kT.T @ qT          (scores transposed, 2 heads row-tiled
                                via tile_position)
  PT      = exp(ST) * maskT    (ACT exp psum->sbuf bf16, then 2x-mode
                                DVE multiplicative bf16 mask; softmax
                                max-subtraction unnecessary: scaled
                                scores are ~N(0,1))
  valsT_aug[65,q] = sum over k-tiles (lhsT=V_aug[k,65], rhs=PT[k,q])
  vals    = valsT * (1/denom)
  out_partial[q,n] = vals.T @ Wo_rows  (bf16 partials; host upcasts)

Scheduling: the PV matmul stream runs one k-tile behind the
scores/exp/mask stream (the PE always has score matmuls in flight
while exp/mask catch up); each head-pair's psum accumulators are
copied to SBUF right after their last accumulation (split ACT/DVE, to
free banks 4-7 for the next pair quickly), and the
reciprocal/broadcast/normalize chain is emitted in chunks woven into
the NEXT pair's k-loop so it never backs up the DVE mask stream; the
[1,S]->[64,S] denominator broadcast runs on the otherwise idle GPSIMD
engine. Per-rep tiles are double-buffered so consecutive reps
pipeline (rep r+1's DMA + QKV matmuls overlap rep r's attention).
"""

import numpy as np

import concourse.bacc as bacc
import concourse.mybir as mybir
import concourse.tile as tile
from concourse import bass_utils
from concourse.alu_op_type import AluOpType

F32 = mybir.dt.float32
BF16 = mybir.dt.bfloat16
AF = mybir.ActivationFunctionType

B, S, D, H, HD = 4, 1024, 1024, 16, 64
NCORES = 8
HPC = 8            # heads per core
HAUG = HD + 1      # 65: V columns per head incl. ones column
VW = HPC * HAUG    # 520


def build_kernel(debug=False, krep=1, **_ignored):
    nc = bacc.Bacc(trn_type="TRN2", target_bir_lowering=False, debug=False,
                   num_devices=NCORES)

    xT = nc.dram_tensor("xT", [D, S], BF16, kind="ExternalInput").ap()
    maskT = nc.dram_tensor("maskT", [S, S], BF16, kind="ExternalInput").ap()
    wq = nc.dram_tensor("wq", [D, 512], BF16, kind="ExternalInput").ap()
    wk = nc.dram_tensor("wk", [D, 512], BF16, kind="ExternalInput").ap()
    wv = nc.dram_tensor("wv", [D, VW], BF16, kind="ExternalInput").ap()
    bq = nc.dram_tensor("bq", [512], F32, kind="ExternalInput").ap()
    bk = nc.dram_tensor("bk", [512], F32, kind="ExternalInput").ap()
    bvb_d = nc.dram_tensor("bvb", [128, VW], F32, kind="ExternalInput").ap()
    wo = nc.dram_tensor("wo", [512, S], BF16, kind="ExternalInput").ap()
    out = nc.dram_tensor("out", [S, S], BF16, kind="ExternalOutput").ap()
    if debug:
        d_q = nc.dram_tensor("d_q", [512, S], F32, kind="ExternalOutput").ap()
        d_k = nc.dram_tensor("d_k", [512, S], F32, kind="ExternalOutput").ap()
        d_vals = nc.dram_tensor("d_vals", [512, S], F32, kind="ExternalOutput").ap()

    PTAG = [f"T{i}" for i in range(8)]   # eight 1-bank psum slots

    with tile.TileContext(nc) as tc:
        with (
            tc.tile_pool(name="persist", bufs=1) as P,
            tc.tile_pool(name="ring", bufs=2) as R,
            tc.tile_pool(name="psum", bufs=1, space="PSUM") as PP,
        ):
            def ptile(shape, idx, name):
                return PP.tile(shape, F32, tag=PTAG[idx & 7],
                               name=f"{name}{idx & 7}")

            # ---- persistent constants ----
            bq_t = P.tile([128, 4], F32, tag="bq", name="bq")
            bk_t = P.tile([128, 4], F32, tag="bk", name="bk")
            bvb = P.tile([128, VW], F32, tag="bvb", name="bvb")
            nc.sync.dma_start(bq_t[:], bq.rearrange("(t p) -> p t", p=128))
            nc.sync.dma_start(bk_t[:], bk.rearrange("(t p) -> p t", p=128))
            nc.sync.dma_start(bvb[:], bvb_d)

            def alloc_rep():
                t = {}
                t["mN"] = [R.tile([128, S], BF16, tag=f"mN{i}", name=f"mN{i}",
                                  bufs=1) for i in range(8)]
                t["xT"] = [R.tile([128, S], BF16, tag=f"xT{i}", name=f"xT{i}")
                           for i in range(8)]
                t["wq"] = [R.tile([128, 512], BF16, tag=f"wq{i}",
                                  name=f"wq{i}") for i in range(8)]
                t["wk"] = [R.tile([128, 512], BF16, tag=f"wk{i}",
                                  name=f"wk{i}") for i in range(8)]
                t["wv"] = [R.tile([128, VW], BF16, tag=f"wv{i}",
                                  name=f"wv{i}") for i in range(8)]
                t["wo"] = [R.tile([128, S], BF16, tag=f"wo{i}",
                                  name=f"wo{i}", bufs=1) for i in range(4)]
                t["qT"] = [R.tile([128, S], BF16, tag=f"qT{i}",
                                  name=f"qT{i}") for i in range(4)]
                t["kT"] = [R.tile([128, S], BF16, tag=f"kT{i}",
                                  name=f"kT{i}") for i in range(4)]
                t["vA"] = [R.tile([128, VW], BF16, tag=f"vA{i}",
                                  name=f"vA{i}") for i in range(8)]
                t["vals"] = [R.tile([128, S], BF16, tag=f"vals{i}",
                                    name=f"vals{i}", bufs=1)
                             for i in range(4)]
                return t

            def emit_dmas(t):
                # xT+wv first: the first woven V-projection chunks need
                # them; masks last (only needed by that rep's attention)
                for i in range(8):
                    rs = slice(i * 128, (i + 1) * 128)
                    nc.sync.dma_start(t["xT"][i][:], xT[rs, :])
                for i in range(8):
                    rs = slice(i * 128, (i + 1) * 128)
                    nc.sync.dma_start(t["wv"][i][:], wv[rs, :])
                for i in range(8):
                    rs = slice(i * 128, (i + 1) * 128)
                    nc.sync.dma_start(t["wq"][i][:], wq[rs, :])
                    nc.sync.dma_start(t["wk"][i][:], wk[rs, :])
                for i in range(4):
                    nc.sync.dma_start(t["wo"][i][:],
                                      wo[i * 128:(i + 1) * 128, :])
                for i in range(8):
                    rs = slice(i * 128, (i + 1) * 128)
                    nc.sync.dma_start(t["mN"][i][:], maskT[rs, :])

            def stage1_chunks(t, tag_iter):
                """QKV projection as 32 independent closures, each one
                psum-accumulation group on a reserved bank (tags T2/T3),
                so they can be woven into the previous rep's exp-paced
                attention loop to fill PE bubbles."""
                chunks = []

                def v_group(st, h2):
                    def go():
                        cs = slice(h2 * 260, (h2 + 1) * 260)
                        ts_ = slice(st * 128, (st + 1) * 128)
                        pv = ptile([128, 260], next(tag_iter), "pv")
                        with nc.allow_low_precision(reason="matmul feed"):
                            for i in range(8):
                                nc.tensor.matmul(pv[:], t["xT"][i][:, ts_],
                                                 t["wv"][i][:, cs],
                                                 start=(i == 0),
                                                 stop=(i == 7))
                            nc.vector.tensor_tensor(t["vA"][st][:, cs],
                                                    pv[:], bvb[:, cs],
                                                    AluOpType.add)
                    return go

                def qk_group(which, tt, sh):
                    def go():
                        cs = slice(tt * 128, (tt + 1) * 128)
                        ss = slice(sh * 512, (sh + 1) * 512)
                        w = t["wq"] if which == "q" else t["wk"]
                        dst = t["qT"] if which == "q" else t["kT"]
                        bias = bq_t if which == "q" else bk_t
                        pq = ptile([128, 512], next(tag_iter), "pq")
                        with nc.allow_low_precision(reason="matmul feed"):
                            for i in range(8):
                                nc.tensor.matmul(pq[:], w[i][:, cs],
                                                 t["xT"][i][:, ss],
                                                 start=(i == 0),
                                                 stop=(i == 7))
                            nc.vector.tensor_scalar(dst[tt][:, ss], pq[:],
                                                    bias[:, tt:tt + 1],
                                                    None, AluOpType.add)
                    return go

                for st in range(8):
                    for h2 in range(2):
                        chunks.append(v_group(st, h2))
                for tt in range(4):
                    for sh in range(2):
                        chunks.append(qk_group("q", tt, sh))
                        chunks.append(qk_group("k", tt, sh))
                return chunks

            def tag23():
                i = 0
                while True:
                    yield 2 + (i & 1)
                    i += 1

            def tag07():
                i = 0
                while True:
                    yield i & 7
                    i += 1

            # ---- software-pipelined rep loop: rep r's attention weaves
            # in rep r+1's QKV projection chunks (psum tags T2/T3) so the
            # exp-paced attention phase keeps the PE busy ----
            den = R.tile([1, S], F32, tag="den", name="den", bufs=1)
            rec = R.tile([1, S], F32, tag="rec", name="rec", bufs=1)

            def norm_chunks(t, p, vraw, bcs_l):
                """Pair p's normalization, split into 4 closures woven
                into the next pair's k-loop so the DVE mask stream never
                backs up: per hh, (stage den f32 + reciprocal + GPSIMD
                broadcast), then the multiplies."""
                def chain(hh):
                    for qh in range(2):
                        qs = slice(qh * 512, (qh + 1) * 512)
                        nc.vector.tensor_copy(den[0:1, qs],
                                              vraw[hh][qh][64:65, :])
                    nc.vector.reciprocal_approx_fast(rec[:], den[:])
                    for qh in range(2):
                        qs = slice(qh * 512, (qh + 1) * 512)
                        nc.gpsimd.partition_broadcast(
                            bcs_l[2 * hh + qh][:], rec[0:1, qs])

                def mults(hh):
                    with nc.allow_low_precision(reason="matmul feed"):
                        for qh in range(2):
                            qs = slice(qh * 512, (qh + 1) * 512)
                            nc.vector.tensor_tensor(
                                t["vals"][p][hh * 64:(hh + 1) * 64, qs],
                                vraw[hh][qh][0:64, :],
                                bcs_l[2 * hh + qh][:], AluOpType.mult)

                return [lambda: chain(0), lambda: mults(0),
                        lambda: chain(1), lambda: mults(1)]

            def attention(t, weave):
                """Attention stage for this rep's tiles. `weave` is a list
                of closures (next rep's stage-1 groups) consumed one per
                k-tile slot to fill PE bubbles."""
                pending = []
                qT, kT, vA, mN = t["qT"], t["kT"], t["vA"], t["mN"]
                for p in range(4):   # head pair
                    vpsf = [[ptile([HAUG, 512], 4 + 2 * hh + qh, "vps")
                             for qh in range(2)] for hh in range(2)]
                    LAG = 2          # PV runs LAG k-tiles behind scores
                    pts = {}
                    for kt in range(8 + LAG):
                        if weave and p >= 1:
                            weave.pop(0)()
                        ks = slice(kt * 128, (kt + 1) * 128)
                        for qh in range(2):
                            qs = slice(qh * 512, (qh + 1) * 512)
                            for hh in range(2):
                                h = 2 * p + hh
                                ds = slice(hh * 64, (hh + 1) * 64)
                                if kt < 8:
                                    stp = ptile([128, 512], 2 * hh + qh,
                                                "stp")
                                    nc.tensor.matmul(stp[:], kT[p][ds, ks],
                                                     qT[p][ds, qs],
                                                     start=True, stop=True,
                                                     tile_position=(hh * 64,
                                                                    0))
                                    pt = R.tile([128, 512], BF16, tag="pt",
                                                name="pt", bufs=12)
                                    nc.scalar.activation(pt[:], stp[:],
                                                         AF.Exp)
                                    with nc.allow_low_precision(
                                            reason="matmul feed"):
                                        nc.vector.tensor_tensor(
                                            pt[:], pt[:], mN[kt][:, qs],
                                            AluOpType.mult)
                                    pts[(kt, qh, hh)] = pt
                                if kt >= LAG:
                                    nc.tensor.matmul(
                                        vpsf[hh][qh][:],
                                        vA[kt - LAG][:,
                                                     h * HAUG:(h + 1) * HAUG],
                                        pts.pop((kt - LAG, qh, hh)),
                                        start=(kt == LAG),
                                        stop=(kt == 7 + LAG))
                        # weave the previous pair's normalization in
                        if kt in (2, 4, 6, 8) and pending:
                            pending.pop(0)()

                    # Copy-first: dump each vpsf psum tile (incl. den row
                    # 64) to SBUF bf16 immediately so banks 4-7 free up
                    # for the next pair; copies split ACT/DVE to halve
                    # the engine-FIFO insertion delay.
                    vraw = [[R.tile([HAUG, 512], BF16, tag=f"vraw{2*hh+qh}",
                                    name="vraw", bufs=2)
                             for qh in range(2)] for hh in range(2)]
                    with nc.allow_low_precision(reason="matmul feed"):
                        for qh in range(2):
                            nc.scalar.activation(vraw[0][qh][:],
                                                 vpsf[0][qh][:], AF.Identity)
                            nc.vector.tensor_copy(vraw[1][qh][:],
                                                  vpsf[1][qh][:])
                    bcs_l = [R.tile([64, 512], F32, tag="bcs", name="bcs",
                                    bufs=4) for _ in range(4)]
                    if p == 3:
                        # Last pair: no next k-loop to weave into; stage
                        # the dens on the now-idle ACT engine straight
                        # from PSUM and run the chains back to back.
                        for hh in range(2):
                            for qh in range(2):
                                qs = slice(qh * 512, (qh + 1) * 512)
                                nc.scalar.activation(den[0:1, qs],
                                                     vpsf[hh][qh][64:65, :],
                                                     AF.Identity)
                            nc.vector.reciprocal_approx_fast(rec[:], den[:])
                            for qh in range(2):
                                qs = slice(qh * 512, (qh + 1) * 512)
                                nc.gpsimd.partition_broadcast(
                                    bcs_l[2 * hh + qh][:], rec[0:1, qs])
                            with nc.allow_low_precision(
                                    reason="matmul feed"):
                                for qh in range(2):
                                    qs = slice(qh * 512, (qh + 1) * 512)
                                    nc.vector.tensor_tensor(
                                        t["vals"][p][hh * 64:(hh + 1) * 64,
                                                     qs],
                                        vraw[hh][qh][0:64, :],
                                        bcs_l[2 * hh + qh][:],
                                        AluOpType.mult)
                    else:
                        pending.extend(norm_chunks(t, p, vraw, bcs_l))

            def outproj(t, weave):
                """Output projection in waves of 3/3/2 qt over psum tags
                T2..T7; within each wave the pi<3 matmuls come first so
                head-pair 3's normalization tail is covered. Leftover
                weave chunks fill the gaps."""
                vals, wo_t = t["vals"], t["wo"]
                for wave, qts in enumerate(([0, 1, 2], [3, 4, 5], [6, 7])):
                    po_w = {}
                    for wi, qt in enumerate(qts):
                        qs = slice(qt * 128, (qt + 1) * 128)
                        for nh in range(2):
                            ns = slice(nh * 512, (nh + 1) * 512)
                            po = ptile([128, 512], 2 + 2 * wi + nh, "po")
                            po_w[(qt, nh)] = po
                            for pi in range(3):
                                nc.tensor.matmul(po[:], vals[pi][:, qs],
                                                 wo_t[pi][:, ns],
                                                 start=(pi == 0), stop=False)
                    if weave:
                        weave.pop(0)()
                    for qt in qts:
                        qs = slice(qt * 128, (qt + 1) * 128)
                        ot = R.tile([128, S], BF16, tag="ot", name="ot",
                                    bufs=2)
                        for nh in range(2):
                            ns = slice(nh * 512, (nh + 1) * 512)
                            po = po_w[(qt, nh)]
                            nc.tensor.matmul(po[:], vals[3][:, qs],
                                             wo_t[3][:, ns],
                                             start=False, stop=True)
                            with nc.allow_low_precision(
                                    reason="matmul feed"):
                                nc.scalar.activation(ot[:, ns], po[:],
                                                     AF.Identity)
                        nc.sync.dma_start(out[qs, :], ot[:])

            # prologue: rep 0's tiles + full stage 1 inline
            cur = alloc_rep()
            emit_dmas(cur)
            for c in stage1_chunks(cur, tag07()):
                c()
            for rep in range(krep):
                if rep + 1 < krep:
                    nxt = alloc_rep()
                    emit_dmas(nxt)
                    weave = stage1_chunks(nxt, tag23())
                else:
                    nxt, weave = None, []
                attention(cur, weave)
                if debug:
                    for pi in range(4):
                        nc.sync.dma_start(
                            d_vals[pi * 128:(pi + 1) * 128, 0:512],
                            cur["vals"][pi][:].bitcast(F32))
                    for tt in range(4):
                        nc.sync.dma_start(d_q[tt * 128:(tt + 1) * 128, 0:512],
                                          cur["qT"][tt][:].bitcast(F32))
                        nc.sync.dma_start(d_k[tt * 128:(tt + 1) * 128, 0:512],
                                          cur["kT"][tt][:].bitcast(F32))
                outproj(cur, weave)
                while weave:
                    weave.pop(0)()
                cur = nxt

    nc.compile()
    return nc


_NC_CACHE = {}


def _get_nc():
    if "nc" not in _NC_CACHE:
        _NC_CACHE["nc"] = build_kernel()
    return _NC_CACHE["nc"]


def _bf16(a):
    import jax.numpy as jnp
    return np.asarray(jnp.asarray(a, dtype=jnp.bfloat16))


def shard_inputs(x, mask, Wqkv, bqkv, Wo, bo):
    """Per-core input dicts. Layout/slicing/dtype staging only."""
    x = np.asarray(x, dtype=np.float32)
    mask = np.asarray(mask, dtype=np.int32)
    Wqkv = np.asarray(Wqkv, dtype=np.float32)
    bqkv = np.asarray(bqkv, dtype=np.float32)
    Wo = np.asarray(Wo, dtype=np.float32)

    scale = 1.0 / np.sqrt(HD)
    Wr = Wqkv.reshape(D, H, 3, HD)
    br = bqkv.reshape(H, 3, HD)
    in_maps = []
    for c in range(NCORES):
        b, g = c // 2, c % 2
        hs = slice(g * HPC, (g + 1) * HPC)
        wv_aug = np.zeros((D, HPC, HAUG), dtype=np.float32)
        wv_aug[:, :, :HD] = Wr[:, hs, 2, :]
        bv_aug = np.zeros((HPC, HAUG), dtype=np.float32)
        bv_aug[:, :HD] = br[hs, 2, :]
        bv_aug[:, HD] = 1.0
        bvb = np.tile(bv_aug.reshape(1, VW), (128, 1))
        in_maps.append({
            "xT": _bf16(x[b].T),
            "maskT": _bf16(mask[b].T),
            "wq": _bf16(Wr[:, hs, 0, :].reshape(D, 512) * scale),
            "wk": _bf16(Wr[:, hs, 1, :].reshape(D, 512)),
            "wv": _bf16(wv_aug.reshape(D, VW)),
            "bq": np.ascontiguousarray(br[hs, 0, :].reshape(512) * scale),
            "bk": np.ascontiguousarray(br[hs, 1, :].reshape(512)),
            "bvb": np.ascontiguousarray(bvb),
            "wo": _bf16(Wo[g * 512:(g + 1) * 512, :]),
        })
    return in_maps


def combine_outputs(results, bo):
    bo = np.asarray(bo, dtype=np.float32)
    out = np.empty((B, S, D), dtype=np.float32)
    for b in range(B):
        out[b] = (np.asarray(results[2 * b]["out"], dtype=np.float32)
                  + np.asarray(results[2 * b + 1]["out"], dtype=np.float32)
                  + bo)
    return out


def kernel(x, mask, Wqkv, bqkv, Wo, bo):
    nc = _get_nc()
    in_maps = shard_inputs(x, mask, Wqkv, bqkv, Wo, bo)
    res = bass_utils.run_bass_kernel_spmd(nc, in_maps,
                                          core_ids=list(range(NCORES)))
    return combine_outputs(res.results, bo)
